# revision 14
# baseline (speedup 1.0000x reference)
"""ChannelSymmetry kernel for Trainium2 (8 NeuronCores, SPMD data-parallel).

Problem: X [128, 64, 8000] f32, swap_mask [128, 16] bool. For each batch b and
channel pair p (channels 2p, 2p+1; p < 16, i.e. channels 0..31), swap the two
channel rows iff swap_mask[b, p]. Channels 32..63 pass through unchanged.

Design: the permutation is runtime data, so it cannot live in compile-time DMA
access patterns. The host turns swap_mask into per-row source indices; the
device does an indirect-DMA row gather (each row = 32KB contiguous, full DMA
efficiency) from HBM into SBUF, then a regular store back to HBM. Pure DMA,
no compute engines — this is a memory-roofline problem.

Sharding: pure data parallel over the batch axis, 16 batches per core.
"""

import contextlib
import sys

import numpy as np

for _p in ("/opt/trn_rl_repo", "/opt/pypackages"):
    if _p not in sys.path:
        sys.path.append(_p)

import concourse.bass as bass
import concourse.mybir as mybir
import concourse.tile as tile
from concourse.bass_utils import run_bass_kernel_spmd

B, C, T = 128, 64, 8000
M = 8            # cores
BL = B // M      # batches per core
ROWS = BL * C    # rows per core (viewing X_shard as [ROWS, T])
P = 128          # SBUF partitions / rows per chunk


def build_bass(rows=ROWS, t=T, nbuf=3):
    """Per-core program: for each chunk of 128 rows, indirect-gather the
    permuted source rows from HBM into SBUF, then store contiguously.

    Raw bass (no Tile): walrus only allows one sync-wait per DMA
    instruction, so waits must be standalone sequencer instructions.
    gpsimd (SWDGE) issues the gathers; sync (HWDGE) issues the stores;
    two semaphores ping-pong the nbuf SBUF slots between them.
    """
    nchunk = rows // P
    nc = bass.Bass()
    x = nc.dram_tensor("x", [rows, t], mybir.dt.float32, kind="ExternalInput")
    idx = nc.dram_tensor("idx", [P, nchunk], mybir.dt.int32, kind="ExternalInput")
    y = nc.dram_tensor("y", [rows, t], mybir.dt.float32, kind="ExternalOutput")

    with contextlib.ExitStack() as ctx:
        idx_t = ctx.enter_context(
            nc.sbuf_tensor("idx_t", [P, nchunk], mybir.dt.int32)
        )
        bufs = [
            ctx.enter_context(nc.sbuf_tensor(f"buf{i}", [P, t], mybir.dt.float32))
            for i in range(nbuf)
        ]
        i_sem = ctx.enter_context(nc.semaphore(name="i_sem"))
        g_sems = [
            ctx.enter_context(nc.semaphore(name=f"g_sem{i}")) for i in range(nbuf)
        ]
        s_sems = [
            ctx.enter_context(nc.semaphore(name=f"s_sem{i}")) for i in range(nbuf)
        ]
        block = ctx.enter_context(nc.Block())

        @block.gpsimd
        def _(g):
            g.dma_start(out=idx_t[:], in_=idx[:]).then_inc(i_sem, 16)
            g.wait_ge(i_sem, 16)
            for ci in range(nchunk):
                sl, rnd = ci % nbuf, ci // nbuf
                if rnd > 0:
                    # slot free once its previous store completed
                    g.wait_ge(s_sems[sl], rnd * 16)
                g.indirect_dma_start(
                    out=bufs[sl][:],
                    out_offset=None,
                    in_=x[:],
                    in_offset=bass.IndirectOffsetOnAxis(
                        ap=idx_t[:, ci : ci + 1], axis=0
                    ),
                ).then_inc(g_sems[sl], 16)

        @block.sync
        def _(s):
            for ci in range(nchunk):
                sl, rnd = ci % nbuf, ci // nbuf
                s.wait_ge(g_sems[sl], (rnd + 1) * 16)
                s.dma_start(
                    out=y[ci * P : (ci + 1) * P, :], in_=bufs[sl][:]
                ).then_inc(s_sems[sl], 16)
            # drain: every slot's stores complete before kernel end
            for sl in range(nbuf):
                nstores = (nchunk - sl + nbuf - 1) // nbuf
                if nstores > 0:
                    s.wait_ge(s_sems[sl], nstores * 16)

    return nc


def build_bass_v2(bl=BL, c=C, t=T, nbuf=3):
    """v2: only the 32 swappable channels go through the SBUF gather+store
    path; the 32 pass-through channels move as direct DRAM->DRAM copies on
    the ACT HWDGE ring. Stream traffic drops from 2x to 1.5x of data size
    and spreads evenly over the three DMA rings (Pool/SP/ACT).
    """
    assert c == 64
    half = c // 2
    rows = bl * c
    grows = bl * half          # gathered rows (channels 0..31 of each batch)
    nchunk = grows // P        # 4 batches per chunk
    assert grows % P == 0
    bpc = P // half            # batches per gather chunk (=4)
    nc = bass.Bass()
    x = nc.dram_tensor("x", [bl, c, t], mybir.dt.float32, kind="ExternalInput")
    idx = nc.dram_tensor("idx", [P, nchunk], mybir.dt.int32, kind="ExternalInput")
    y = nc.dram_tensor("y", [bl, c, t], mybir.dt.float32, kind="ExternalOutput")
    x_flat = x.rearrange("b c t -> (b c) t")

    with contextlib.ExitStack() as ctx:
        idx_t = ctx.enter_context(
            nc.sbuf_tensor("idx_t", [P, nchunk], mybir.dt.int32)
        )
        bufs = [
            ctx.enter_context(nc.sbuf_tensor(f"buf{i}", [P, t], mybir.dt.float32))
            for i in range(nbuf)
        ]
        i_sem = ctx.enter_context(nc.semaphore(name="i_sem"))
        g_sems = [
            ctx.enter_context(nc.semaphore(name=f"g_sem{i}")) for i in range(nbuf)
        ]
        s_sems = [
            ctx.enter_context(nc.semaphore(name=f"s_sem{i}")) for i in range(nbuf)
        ]
        d_sem = ctx.enter_context(nc.semaphore(name="d_sem"))
        block = ctx.enter_context(nc.Block())

        @block.scalar
        def _(a):
            # independent pass-through copies, one per gather-chunk's batches
            for ci in range(nchunk):
                a.dma_start(
                    out=y[ci * bpc : (ci + 1) * bpc, half:c, :],
                    in_=x[ci * bpc : (ci + 1) * bpc, half:c, :],
                ).then_inc(d_sem, 16)
            a.wait_ge(d_sem, nchunk * 16)

        @block.gpsimd
        def _(g):
            g.dma_start(out=idx_t[:], in_=idx[:]).then_inc(i_sem, 16)
            g.wait_ge(i_sem, 16)
            for ci in range(nchunk):
                sl, rnd = ci % nbuf, ci // nbuf
                if rnd > 0:
                    g.wait_ge(s_sems[sl], rnd * 16)
                g.indirect_dma_start(
                    out=bufs[sl][:],
                    out_offset=None,
                    in_=x_flat[:],
                    in_offset=bass.IndirectOffsetOnAxis(
                        ap=idx_t[:, ci : ci + 1], axis=0
                    ),
                ).then_inc(g_sems[sl], 16)

        @block.sync
        def _(s):
            for ci in range(nchunk):
                sl, rnd = ci % nbuf, ci // nbuf
                s.wait_ge(g_sems[sl], (rnd + 1) * 16)
                s.dma_start(
                    out=y[ci * bpc : (ci + 1) * bpc, 0:half, :], in_=bufs[sl][:]
                ).then_inc(s_sems[sl], 16)
            for sl in range(nbuf):
                nstores = (nchunk - sl + nbuf - 1) // nbuf
                if nstores > 0:
                    s.wait_ge(s_sems[sl], nstores * 16)

    return nc


def build_bass_v4(bl=BL, c=C, t=T, nbuf=3):
    """v4: true in-place. `y` arrives pre-initialized with this core's X
    shard (donated PJRT buffer). Only channels 0..31 move: indirect-gather
    the permuted rows out of y itself into SBUF, then store them back.
    Channels 32..63 are never touched. Per-chunk pipelining is safe: chunk
    ci's gather reads exactly the rows chunk ci's store later writes, and
    different chunks touch disjoint row sets.
    """
    assert c == 64
    half = c // 2
    nchunk = bl * half // P    # gather chunks (4 batches each)
    bpc = P // half
    nc = bass.Bass()
    idx = nc.dram_tensor("idx", [P, nchunk], mybir.dt.int32, kind="ExternalInput")
    y = nc.dram_tensor("y", [bl, c, t], mybir.dt.float32, kind="ExternalOutput")
    y_flat = y.rearrange("b c t -> (b c) t")

    with contextlib.ExitStack() as ctx:
        idx_t = ctx.enter_context(
            nc.sbuf_tensor("idx_t", [P, nchunk], mybir.dt.int32)
        )
        bufs = [
            ctx.enter_context(nc.sbuf_tensor(f"buf{i}", [P, t], mybir.dt.float32))
            for i in range(nbuf)
        ]
        i_sem = ctx.enter_context(nc.semaphore(name="i_sem"))
        g_sems = [
            ctx.enter_context(nc.semaphore(name=f"g_sem{i}")) for i in range(nbuf)
        ]
        s_sems = [
            ctx.enter_context(nc.semaphore(name=f"s_sem{i}")) for i in range(nbuf)
        ]
        block = ctx.enter_context(nc.Block())

        @block.gpsimd
        def _(g):
            g.dma_start(out=idx_t[:], in_=idx[:]).then_inc(i_sem, 16)
            g.wait_ge(i_sem, 16)
            for ci in range(nchunk):
                sl, rnd = ci % nbuf, ci // nbuf
                if rnd > 0:
                    g.wait_ge(s_sems[sl], rnd * 16)
                g.indirect_dma_start(
                    out=bufs[sl][:],
                    out_offset=None,
                    in_=y_flat[:],
                    in_offset=bass.IndirectOffsetOnAxis(
                        ap=idx_t[:, ci : ci + 1], axis=0
                    ),
                ).then_inc(g_sems[sl], 16)

        @block.sync
        def _(s):
            for ci in range(nchunk):
                sl, rnd = ci % nbuf, ci // nbuf
                s.wait_ge(g_sems[sl], (rnd + 1) * 16)
                s.dma_start(
                    out=y[ci * bpc : (ci + 1) * bpc, 0:half, :], in_=bufs[sl][:]
                ).then_inc(s_sems[sl], 16)
            for sl in range(nbuf):
                nstores = (nchunk - sl + nbuf - 1) // nbuf
                if nstores > 0:
                    s.wait_ge(s_sems[sl], nstores * 16)

    return nc


def build_bass_v5(bl=BL, c=C, t=T, nbuf=3):
    """v5: in-place like v4, but every DRAM-side AP is 2D contiguous
    (3D strided DRAM APs measured ~4.5x slower on HWDGE). Each gather
    chunk's 4 batches are stored as 4 separate 1MB contiguous stores.
    idx loads via HWDGE (sync) to shave SWDGE startup.
    """
    assert c == 64
    half = c // 2
    nchunk = bl * half // P    # 4 chunks of 4 batches
    bpc = P // half            # batches per chunk
    nc = bass.Bass()
    idx = nc.dram_tensor("idx", [P, nchunk], mybir.dt.int32, kind="ExternalInput")
    y = nc.dram_tensor("y", [bl, c, t], mybir.dt.float32, kind="ExternalOutput")
    y_flat = y.rearrange("b c t -> (b c) t")

    with contextlib.ExitStack() as ctx:
        idx_t = ctx.enter_context(
            nc.sbuf_tensor("idx_t", [P, nchunk], mybir.dt.int32)
        )
        bufs = [
            ctx.enter_context(nc.sbuf_tensor(f"buf{i}", [P, t], mybir.dt.float32))
            for i in range(nbuf)
        ]
        i_sem = ctx.enter_context(nc.semaphore(name="i_sem"))
        g_sems = [
            ctx.enter_context(nc.semaphore(name=f"g_sem{i}")) for i in range(nbuf)
        ]
        s_sems = [
            ctx.enter_context(nc.semaphore(name=f"s_sem{i}")) for i in range(nbuf)
        ]
        block = ctx.enter_context(nc.Block())

        @block.gpsimd
        def _(g):
            g.wait_ge(i_sem, 16)
            for ci in range(nchunk):
                sl, rnd = ci % nbuf, ci // nbuf
                if rnd > 0:
                    # slot free once its previous 4 stores completed
                    g.wait_ge(s_sems[sl], rnd * 64)
                g.indirect_dma_start(
                    out=bufs[sl][:],
                    out_offset=None,
                    in_=y_flat[:],
                    in_offset=bass.IndirectOffsetOnAxis(
                        ap=idx_t[:, ci : ci + 1], axis=0
                    ),
                ).then_inc(g_sems[sl], 16)

        @block.sync
        def _(s):
            s.dma_start(out=idx_t[:], in_=idx[:]).then_inc(i_sem, 16)
            for ci in range(nchunk):
                sl, rnd = ci % nbuf, ci // nbuf
                s.wait_ge(g_sems[sl], (rnd + 1) * 16)
                for j in range(bpc):
                    row0 = (ci * bpc + j) * c
                    s.dma_start(
                        out=y_flat[row0 : row0 + half, :],
                        in_=bufs[sl][j * half : (j + 1) * half, :],
                    ).then_inc(s_sems[sl], 16)
            for sl in range(nbuf):
                nstores = (nchunk - sl + nbuf - 1) // nbuf
                if nstores > 0:
                    s.wait_ge(s_sems[sl], nstores * 64)

    return nc


def build_bass_v6(bl=BL, c=C, t=T, nbuf=3):
    """v6: in-place + dma_gather (TIE-accelerated descriptor gen, ~0.34ns/desc
    vs ~127ns for indirect_dma_start) + stride-4 partition interleave so each
    batch's 1MB contiguous store spans all 16 SDMA engines.

    Gather position i of chunk ci = (batch i%4, channel i//4), so store j
    reads SBUF partitions j::4 and writes one contiguous 32-row block.
    """
    assert c == 64
    half = c // 2
    nchunk = bl * half // P
    bpc = P // half
    nc = bass.Bass()
    idx = nc.dram_tensor(
        "idx", [P, nchunk * 8], mybir.dt.int16, kind="ExternalInput"
    )
    y = nc.dram_tensor("y", [bl, c, t], mybir.dt.float32, kind="ExternalOutput")
    y_flat = y.rearrange("b c t -> (b c) t")

    with contextlib.ExitStack() as ctx:
        idx_t = ctx.enter_context(
            nc.sbuf_tensor("idx_t", [P, nchunk * 8], mybir.dt.int16)
        )
        bufs = [
            ctx.enter_context(
                nc.sbuf_tensor(f"buf{i}", [P, 1, t], mybir.dt.float32)
            )
            for i in range(nbuf)
        ]
        i_sem = ctx.enter_context(nc.semaphore(name="i_sem"))
        g_sems = [
            ctx.enter_context(nc.semaphore(name=f"g_sem{i}")) for i in range(nbuf)
        ]
        s_sems = [
            ctx.enter_context(nc.semaphore(name=f"s_sem{i}")) for i in range(nbuf)
        ]
        block = ctx.enter_context(nc.Block())

        @block.gpsimd
        def _(g):
            from concourse import library_config

            g.load_library(library_config.attnmlp)
            g.wait_ge(i_sem, 16)
            for ci in range(nchunk):
                sl, rnd = ci % nbuf, ci // nbuf
                if rnd > 0:
                    g.wait_ge(s_sems[sl], rnd * 64)
                g.dma_gather(
                    bufs[sl][:],
                    y_flat[:],
                    idx_t[:, ci * 8 : (ci + 1) * 8],
                    P,
                    P,
                    t,
                ).then_inc(g_sems[sl], 16)

        @block.sync
        def _(s):
            s.dma_start(out=idx_t[:], in_=idx[:]).then_inc(i_sem, 16)
            for ci in range(nchunk):
                sl, rnd = ci % nbuf, ci // nbuf
                s.wait_ge(g_sems[sl], (rnd + 1) * 16)
                for j in range(bpc):
                    row0 = (ci * bpc + j) * c
                    s.dma_start(
                        out=y_flat[row0 : row0 + half, :],
                        in_=bufs[sl][j : P : bpc, 0, :],
                    ).then_inc(s_sems[sl], 16)
            for sl in range(nbuf):
                nstores = (nchunk - sl + nbuf - 1) // nbuf
                if nstores > 0:
                    s.wait_ge(s_sems[sl], nstores * 64)

    return nc


def build_bass_v7(nchunk, nbuf, bl=BL, c=C, t=T):
    """v7: in-place, minimal traffic. Only rows whose pair actually swaps
    move: indirect-gather each swapped row's partner into SBUF, then
    indirect-scatter it back to the swapped row's slot. Cores with fewer
    swaps than the SPMD-wide max pad their index columns with OOB entries
    (idx > bounds_check, oob_is_err=False) which generate no descriptors.

    idx layout: [128, 2*nchunk] int32; col 2ci = gather (partner) rows,
    col 2ci+1 = scatter (destination) rows for chunk ci. Both rows of a
    pair sit in the same chunk, so pipelined chunks touch disjoint rows.
    """
    rows = bl * c
    nc = bass.Bass()
    idx = nc.dram_tensor(
        "idx", [P, 2 * nchunk], mybir.dt.int32, kind="ExternalInput"
    )
    y = nc.dram_tensor("y", [bl, c, t], mybir.dt.float32, kind="ExternalOutput")
    y_flat = y.rearrange("b c t -> (b c) t")

    with contextlib.ExitStack() as ctx:
        idx_t = ctx.enter_context(
            nc.sbuf_tensor("idx_t", [P, 2 * nchunk], mybir.dt.int32)
        )
        bufs = [
            ctx.enter_context(nc.sbuf_tensor(f"buf{i}", [P, t], mybir.dt.float32))
            for i in range(nbuf)
        ]
        i_sem = ctx.enter_context(nc.semaphore(name="i_sem"))
        g_sems = [
            ctx.enter_context(nc.semaphore(name=f"g_sem{i}")) for i in range(nbuf)
        ]
        s_sems = [
            ctx.enter_context(nc.semaphore(name=f"s_sem{i}")) for i in range(nbuf)
        ]
        block = ctx.enter_context(nc.Block())

        @block.gpsimd
        def _(g):
            def gather(ci):
                sl = ci % nbuf
                g.indirect_dma_start(
                    out=bufs[sl][:],
                    out_offset=None,
                    in_=y_flat[:],
                    in_offset=bass.IndirectOffsetOnAxis(
                        ap=idx_t[:, 2 * ci : 2 * ci + 1], axis=0
                    ),
                    bounds_check=rows - 1,
                    oob_is_err=False,
                ).then_inc(g_sems[sl], 16)

            def scatter(ci):
                sl = ci % nbuf
                g.wait_ge(g_sems[sl], (ci // nbuf + 1) * 16)
                g.indirect_dma_start(
                    out=y_flat[:],
                    out_offset=bass.IndirectOffsetOnAxis(
                        ap=idx_t[:, 2 * ci + 1 : 2 * ci + 2], axis=0
                    ),
                    in_=bufs[sl][:],
                    in_offset=None,
                    bounds_check=rows - 1,
                    oob_is_err=False,
                ).then_inc(s_sems[sl], 16)

            g.wait_ge(i_sem, 16)
            # software-pipelined: gathers run nbuf-1 chunks ahead of scatters
            for ci in range(nchunk):
                if ci >= nbuf:
                    g.wait_ge(s_sems[ci % nbuf], (ci // nbuf) * 16)
                gather(ci)
                cj = ci - (nbuf - 1)
                if cj >= 0:
                    scatter(cj)
            for cj in range(max(0, nchunk - (nbuf - 1)), nchunk):
                scatter(cj)
            for sl in range(nbuf):
                nst = (nchunk - sl + nbuf - 1) // nbuf
                if nst > 0:
                    g.wait_ge(s_sems[sl], nst * 16)

        @block.sync
        def _(s):
            s.dma_start(out=idx_t[:], in_=idx[:]).then_inc(i_sem, 16)

    return nc


def build_bass_v8(nchunk, nbuf, split, bl=BL, c=C, t=T):
    """v8: v7 with each 32KB row split into `split` sub-row descriptors.
    The SWDGE deals descriptors to the 16 SDMA engines in blocks of 8, so
    smaller descriptors shrink the per-engine granularity (load imbalance
    from partial tail chunks drops from ~10us to ~10/split us).

    idx layout: [128, 2*split*nchunk] int32 into y viewed as
    [(b c split), t/split]. Chunk ci: cols [2s*ci, 2s*ci+s) = gather descs
    (desc j of the chunk feeds buf partition j//s, sub-row j%s), cols
    [2s*ci+s, 2s*ci+2s) = scatter descs.
    """
    s_ = split
    rows = bl * c * s_
    ts = t // s_
    nc = bass.Bass()
    idx = nc.dram_tensor(
        "idx", [P, 2 * s_ * nchunk], mybir.dt.int32, kind="ExternalInput"
    )
    y = nc.dram_tensor("y", [bl, c, t], mybir.dt.float32, kind="ExternalOutput")
    y_sub = y.rearrange("b c (s x) -> (b c s) x", s=s_)

    with contextlib.ExitStack() as ctx:
        idx_t = ctx.enter_context(
            nc.sbuf_tensor("idx_t", [P, 2 * s_ * nchunk], mybir.dt.int32)
        )
        bufs = [
            ctx.enter_context(nc.sbuf_tensor(f"buf{i}", [P, t], mybir.dt.float32))
            for i in range(nbuf)
        ]
        i_sem = ctx.enter_context(nc.semaphore(name="i_sem"))
        g_sems = [
            ctx.enter_context(nc.semaphore(name=f"g_sem{i}")) for i in range(nbuf)
        ]
        s_sems = [
            ctx.enter_context(nc.semaphore(name=f"s_sem{i}")) for i in range(nbuf)
        ]
        block = ctx.enter_context(nc.Block())

        @block.gpsimd
        def _(g):
            def gather(ci):
                sl = ci % nbuf
                a = 2 * s_ * ci
                g.indirect_dma_start(
                    out=bufs[sl][:],
                    out_offset=None,
                    in_=y_sub[:],
                    in_offset=bass.IndirectOffsetOnAxis(
                        ap=idx_t[:, a : a + s_], axis=0
                    ),
                    bounds_check=rows - 1,
                    oob_is_err=False,
                ).then_inc(g_sems[sl], 16)

            def scatter(ci):
                sl = ci % nbuf
                a = 2 * s_ * ci + s_
                g.wait_ge(g_sems[sl], (ci // nbuf + 1) * 16)
                g.indirect_dma_start(
                    out=y_sub[:],
                    out_offset=bass.IndirectOffsetOnAxis(
                        ap=idx_t[:, a : a + s_], axis=0
                    ),
                    in_=bufs[sl][:],
                    in_offset=None,
                    bounds_check=rows - 1,
                    oob_is_err=False,
                ).then_inc(s_sems[sl], 16)

            g.wait_ge(i_sem, 16)
            for ci in range(nchunk):
                if ci >= nbuf:
                    g.wait_ge(s_sems[ci % nbuf], (ci // nbuf) * 16)
                gather(ci)
                cj = ci - (nbuf - 1)
                if cj >= 0:
                    scatter(cj)
            for cj in range(max(0, nchunk - (nbuf - 1)), nchunk):
                scatter(cj)
            for sl in range(nbuf):
                nst = (nchunk - sl + nbuf - 1) // nbuf
                if nst > 0:
                    g.wait_ge(s_sems[sl], nst * 16)

        @block.sync
        def _(s):
            s.dma_start(out=idx_t[:], in_=idx[:]).then_inc(i_sem, 16)

    return nc


def build_bass_v9(nchunk, nbuf, split, bl=BL, c=C, t=T):
    """v9: like v8 but each chunk/direction issues `split` sub-instructions;
    sub-instruction k moves only sub-row k of every row (128 descriptors of
    32000/split bytes, strided a full row apart, so the DGE coalescer cannot
    re-merge them). Engine-dealing quantum drops 8x32KB -> 8x(32KB/split).

    idx layout: [128, 2*split*nchunk]; col 2s*ci+k = gather sub-instr k of
    chunk ci (values src_row*split+k), col 2s*ci+s+k = scatter sub-instr k.
    """
    s_ = split
    rows = bl * c * s_
    ts = t // s_
    nc = bass.Bass()
    idx = nc.dram_tensor(
        "idx", [P, 2 * s_ * nchunk], mybir.dt.int32, kind="ExternalInput"
    )
    y = nc.dram_tensor("y", [bl, c, t], mybir.dt.float32, kind="ExternalOutput")
    y_sub = y.rearrange("b c (s x) -> (b c s) x", s=s_)

    with contextlib.ExitStack() as ctx:
        idx_t = ctx.enter_context(
            nc.sbuf_tensor("idx_t", [P, 2 * s_ * nchunk], mybir.dt.int32)
        )
        bufs = [
            ctx.enter_context(nc.sbuf_tensor(f"buf{i}", [P, t], mybir.dt.float32))
            for i in range(nbuf)
        ]
        i_sem = ctx.enter_context(nc.semaphore(name="i_sem"))
        g_sems = [
            ctx.enter_context(nc.semaphore(name=f"g_sem{i}")) for i in range(nbuf)
        ]
        s_sems = [
            ctx.enter_context(nc.semaphore(name=f"s_sem{i}")) for i in range(nbuf)
        ]
        block = ctx.enter_context(nc.Block())

        @block.gpsimd
        def _(g):
            def gather(ci):
                sl = ci % nbuf
                for k in range(s_):
                    a = 2 * s_ * ci + k
                    g.indirect_dma_start(
                        out=bufs[sl][:, k * ts : (k + 1) * ts],
                        out_offset=None,
                        in_=y_sub[:],
                        in_offset=bass.IndirectOffsetOnAxis(
                            ap=idx_t[:, a : a + 1], axis=0
                        ),
                        bounds_check=rows - 1,
                        oob_is_err=False,
                    ).then_inc(g_sems[sl], 16)

            def scatter(ci):
                sl = ci % nbuf
                g.wait_ge(g_sems[sl], (ci // nbuf + 1) * s_ * 16)
                for k in range(s_):
                    a = 2 * s_ * ci + s_ + k
                    g.indirect_dma_start(
                        out=y_sub[:],
                        out_offset=bass.IndirectOffsetOnAxis(
                            ap=idx_t[:, a : a + 1], axis=0
                        ),
                        in_=bufs[sl][:, k * ts : (k + 1) * ts],
                        in_offset=None,
                        bounds_check=rows - 1,
                        oob_is_err=False,
                    ).then_inc(s_sems[sl], 16)

            g.wait_ge(i_sem, 16)
            for ci in range(nchunk):
                if ci >= nbuf:
                    g.wait_ge(s_sems[ci % nbuf], (ci // nbuf) * s_ * 16)
                gather(ci)
                cj = ci - (nbuf - 1)
                if cj >= 0:
                    scatter(cj)
            for cj in range(max(0, nchunk - (nbuf - 1)), nchunk):
                scatter(cj)
            for sl in range(nbuf):
                nst = (nchunk - sl + nbuf - 1) // nbuf
                if nst > 0:
                    g.wait_ge(s_sems[sl], nst * s_ * 16)

        @block.sync
        def _(s):
            s.dma_start(out=idx_t[:], in_=idx[:]).then_inc(i_sem, 16)

    return nc


def make_in_maps_v9(X, swap_mask, split):
    """Row lists as v7; idx col (2s*ci + dir*s + k) = chunk ci's row
    indices *split + k (identity slot mapping, sub-row k per column)."""
    X = np.asarray(X, dtype=np.float32)
    swap_mask = np.asarray(swap_mask).astype(bool)
    b, c, t = X.shape

    src_lists, dst_lists = [], []
    for m in range(M):
        sm = swap_mask[m * BL : (m + 1) * BL]
        blv, pv = np.nonzero(sm)
        a = (blv * c + 2 * pv).astype(np.int32)
        src = np.empty(2 * a.size, dtype=np.int32)
        dst = np.empty(2 * a.size, dtype=np.int32)
        src[0::2], src[1::2] = a + 1, a
        dst[0::2], dst[1::2] = a, a + 1
        src_lists.append(src)
        dst_lists.append(dst)

    lmax = max(s.size for s in src_lists)
    nchunk = max(1, -(-lmax // P))
    lpad = nchunk * P

    in_maps, init_outs = [], []
    for m in range(M):
        src = np.full(lpad, OOB_PAD, dtype=np.int32)
        dst = np.full(lpad, OOB_PAD, dtype=np.int32)
        src[: src_lists[m].size] = src_lists[m]
        dst[: dst_lists[m].size] = dst_lists[m]
        srcc = src.reshape(nchunk, P)
        dstc = dst.reshape(nchunk, P)
        idxm = np.empty((P, 2 * split * nchunk), dtype=np.int32)
        for ci in range(nchunk):
            for k in range(split):
                idxm[:, 2 * split * ci + k] = srcc[ci] * split + k
                idxm[:, 2 * split * ci + split + k] = dstc[ci] * split + k
        in_maps.append({"idx": np.ascontiguousarray(idxm)})
        init_outs.append({"y": np.ascontiguousarray(X[m * BL : (m + 1) * BL])})
    return in_maps, init_outs, nchunk


def make_in_maps_v8(X, swap_mask, split):
    """Like v7 but indices address sub-rows (row r -> split descs
    r*split+q), interleaved per chunk as [gather s cols][scatter s cols]."""
    X = np.asarray(X, dtype=np.float32)
    swap_mask = np.asarray(swap_mask).astype(bool)
    b, c, t = X.shape

    src_lists, dst_lists = [], []
    for m in range(M):
        sm = swap_mask[m * BL : (m + 1) * BL]
        blv, pv = np.nonzero(sm)
        a = (blv * c + 2 * pv).astype(np.int32)
        src = np.empty(2 * a.size, dtype=np.int32)
        dst = np.empty(2 * a.size, dtype=np.int32)
        src[0::2], src[1::2] = a + 1, a
        dst[0::2], dst[1::2] = a, a + 1
        src_lists.append(src)
        dst_lists.append(dst)

    lmax = max(s.size for s in src_lists)
    nchunk = max(1, -(-lmax // P))
    lpad = nchunk * P

    in_maps, init_outs = [], []
    qoff = np.arange(split, dtype=np.int32)
    for m in range(M):
        src = np.full(lpad, OOB_PAD, dtype=np.int32)
        dst = np.full(lpad, OOB_PAD, dtype=np.int32)
        src[: src_lists[m].size] = src_lists[m]
        dst[: dst_lists[m].size] = dst_lists[m]
        # sub-row descs: [lpad, split]; OOB rows stay OOB (pad*split+q > bound)
        srcq = src[:, None] * split + qoff[None, :]
        dstq = dst[:, None] * split + qoff[None, :]
        # -> [nchunk, P, split] -> idx[p, 2s*ci + q] etc.
        idxm = np.empty((P, 2 * split * nchunk), dtype=np.int32)
        srcq = srcq.reshape(nchunk, P, split)
        dstq = dstq.reshape(nchunk, P, split)
        # slot shuffle: buf slot (p, q) <- entry (p+q)%P, quarter q, so
        # consecutive descriptors touch different DRAM rows and the DGE
        # cannot re-aggregate them into 32KB descriptors
        pidx = (np.arange(P)[:, None] + qoff[None, :]) % P  # [P, split]
        srcq = srcq[:, pidx, qoff[None, :]]
        dstq = dstq[:, pidx, qoff[None, :]]
        for ci in range(nchunk):
            idxm[:, 2 * split * ci : 2 * split * ci + split] = srcq[ci]
            idxm[:, 2 * split * ci + split : 2 * split * (ci + 1)] = dstq[ci]
        in_maps.append({"idx": np.ascontiguousarray(idxm)})
        init_outs.append({"y": np.ascontiguousarray(X[m * BL : (m + 1) * BL])})
    return in_maps, init_outs, nchunk


OOB_PAD = 1 << 20


def make_in_maps_v7(X, swap_mask):
    """Per-core (src, dst) row lists for swapped pairs only, padded with
    OOB entries to the max core's length rounded up to full 128-chunks."""
    X = np.asarray(X, dtype=np.float32)
    swap_mask = np.asarray(swap_mask).astype(bool)
    b, c, t = X.shape

    src_lists, dst_lists = [], []
    for m in range(M):
        sm = swap_mask[m * BL : (m + 1) * BL]  # [BL, 16]
        blv, pv = np.nonzero(sm)
        a = (blv * c + 2 * pv).astype(np.int32)  # even row of each pair
        # entries appended in pair order: (dst=a, src=a+1), (dst=a+1, src=a)
        src = np.empty(2 * a.size, dtype=np.int32)
        dst = np.empty(2 * a.size, dtype=np.int32)
        src[0::2], src[1::2] = a + 1, a
        dst[0::2], dst[1::2] = a, a + 1
        src_lists.append(src)
        dst_lists.append(dst)

    lmax = max(s.size for s in src_lists)
    nchunk = max(1, -(-lmax // P))
    lpad = nchunk * P

    in_maps, init_outs = [], []
    for m in range(M):
        src = np.full(lpad, OOB_PAD, dtype=np.int32)
        dst = np.full(lpad, OOB_PAD, dtype=np.int32)
        n = src_lists[m].size
        nfull = (n // P) * P
        src[:nfull] = src_lists[m][:nfull]
        dst[:nfull] = dst_lists[m][:nfull]
        rem = n - nfull
        if rem:
            # The DGE deals each instruction's descriptor list to the 16
            # engines as equal contiguous position slices (pre-OOB-skip,
            # slice->engine mapping is some fixed permutation). Round the
            # partial chunk's real count up to a multiple of 16 with
            # harmless self-copy entries (rows >= ch32 never swap), then
            # place them at a stride dividing 8 so every slice gets an
            # equal share no matter how slices map to engines.
            remp = min(P, 16 * -(-rem // 16))
            npad = remp - rem
            tail_src = np.concatenate(
                [src_lists[m][nfull:], 32 + np.arange(npad, dtype=np.int32)]
            )
            tail_dst = np.concatenate(
                [dst_lists[m][nfull:], 32 + np.arange(npad, dtype=np.int32)]
            )
            pos = nfull + (np.arange(remp) * P // remp)
            src[pos] = tail_src
            dst[pos] = tail_dst
        # idx[p, 2*ci] = src of entry ci*P+p; idx[p, 2*ci+1] = dst
        idxm = np.empty((P, 2 * nchunk), dtype=np.int32)
        idxm[:, 0::2] = src.reshape(nchunk, P).T
        idxm[:, 1::2] = dst.reshape(nchunk, P).T
        in_maps.append({"idx": np.ascontiguousarray(idxm)})
        init_outs.append({"y": np.ascontiguousarray(X[m * BL : (m + 1) * BL])})
    return in_maps, init_outs, nchunk


def make_in_maps_v6(X, swap_mask):
    X = np.asarray(X, dtype=np.float32)
    swap_mask = np.asarray(swap_mask).astype(bool)
    b, c, t = X.shape
    half = c // 2
    nchunk = BL * half // P
    bpc = P // half

    cidx = np.arange(half, dtype=np.int32)
    mask_c = np.repeat(swap_mask, 2, axis=1)
    perm = np.where(mask_c, cidx[None, :] ^ 1, cidx[None, :]).astype(np.int32)

    in_maps, init_outs = [], []
    for m in range(M):
        pm = perm[m * BL : (m + 1) * BL]  # [BL, 32]
        idx16 = np.zeros((P, nchunk * 8), dtype=np.int16)
        for ci in range(nchunk):
            for i in range(P):
                j, k = i % bpc, i // bpc
                bl_loc = ci * bpc + j
                idx16[i % 16, ci * 8 + i // 16] = bl_loc * c + pm[bl_loc, k]
        in_maps.append({"idx": idx16})
        init_outs.append({"y": np.ascontiguousarray(X[m * BL : (m + 1) * BL])})
    return in_maps, init_outs


def _run_pjrt_with_init(nc, in_maps, init_out_maps, n_cores=M):
    """Execute `nc` via PJRT on n_cores devices, donating PRE-INITIALIZED
    output buffers (instead of bass2jax's zeros) so in-place kernels see
    their starting contents. Mirrors concourse.bass2jax.run_bass_via_pjrt.
    """
    import jax
    from jax.experimental.shard_map import shard_map
    from jax.sharding import Mesh, PartitionSpec

    from concourse import bass2jax as b2j

    b2j.install_neuronx_cc_hook()
    assert nc.dbg_addr is None
    partition_name = (
        nc.partition_id_tensor.name if nc.partition_id_tensor else None
    )

    in_names, out_names, out_avals, out_shapes = [], [], [], []
    for alloc in nc.m.functions[0].allocations:
        if not isinstance(alloc, mybir.MemoryLocationSet):
            continue
        name = alloc.memorylocations[0].name
        if alloc.kind == "ExternalInput":
            if name != partition_name:
                in_names.append(name)
        elif alloc.kind == "ExternalOutput":
            shape = tuple(alloc.tensor_shape)
            dtype = mybir.dt.np(alloc.dtype)
            out_names.append(name)
            out_shapes.append((shape, dtype))
            out_avals.append(jax.core.ShapedArray(shape, dtype))
    n_params = len(in_names)
    n_outs = len(out_names)
    all_in_names = list(in_names) + list(out_names)
    if partition_name is not None:
        all_in_names.append(partition_name)

    donate = tuple(range(n_params, n_params + n_outs))

    def _body(*args):
        operands = list(args)
        if partition_name is not None:
            operands.append(b2j.partition_id_tensor())
        outs = b2j._bass_exec_p.bind(
            *operands,
            out_avals=tuple(out_avals),
            in_names=tuple(all_in_names),
            out_names=tuple(out_names),
            lowering_input_output_aliases=(),
            sim_require_finite=True,
            sim_require_nnan=True,
            nc=nc,
        )
        return tuple(outs)

    devices = jax.devices()[:n_cores]
    assert len(devices) == n_cores
    mesh = Mesh(np.asarray(devices), ("core",))
    in_specs = (PartitionSpec("core"),) * (n_params + n_outs)
    out_specs = (PartitionSpec("core"),) * n_outs
    sharded = jax.jit(
        shard_map(
            _body, mesh=mesh, in_specs=in_specs, out_specs=out_specs,
            check_rep=False,
        ),
        donate_argnums=donate,
        keep_unused=True,
    )
    concat_in = [
        np.concatenate(
            [np.asarray(m[name]) for m in in_maps], axis=0
        )
        for name in in_names
    ]
    concat_init = [
        np.concatenate(
            [np.asarray(m[name]) for m in init_out_maps], axis=0
        )
        for name in out_names
    ]
    out_arrs = sharded(*concat_in, *concat_init)
    return [
        {
            name: np.asarray(out_arrs[i]).reshape(
                n_cores, *out_shapes[i][0]
            )[ci]
            for i, name in enumerate(out_names)
        }
        for ci in range(n_cores)
    ]


def make_in_maps(X, swap_mask):
    X = np.asarray(X, dtype=np.float32)
    swap_mask = np.asarray(swap_mask).astype(bool)
    b, c, t = X.shape

    # Source-channel permutation per batch: perm[b, ch] = channel to read.
    cidx = np.arange(c, dtype=np.int32)
    partner = np.where(cidx < 32, cidx ^ 1, cidx).astype(np.int32)
    mask_c = np.zeros((b, c), dtype=bool)
    mask_c[:, :32] = np.repeat(swap_mask, 2, axis=1)
    perm = np.where(mask_c, partner[None, :], cidx[None, :]).astype(np.int32)

    in_maps = []
    for m in range(M):
        xs = np.ascontiguousarray(X[m * BL : (m + 1) * BL].reshape(BL * c, t))
        pm = perm[m * BL : (m + 1) * BL]  # [BL, c]
        rows = (np.arange(BL, dtype=np.int32)[:, None] * c + pm).reshape(-1)
        # idx[p, chunk] = source row feeding output row chunk*P + p
        idxm = np.ascontiguousarray(rows.reshape(-1, P).T.astype(np.int32))
        in_maps.append({"x": xs, "idx": idxm})
    return in_maps


def make_in_maps_v2(X, swap_mask):
    X = np.asarray(X, dtype=np.float32)
    swap_mask = np.asarray(swap_mask).astype(bool)
    b, c, t = X.shape
    half = c // 2

    # source channel for output channels 0..31 (stays within 0..31)
    cidx = np.arange(half, dtype=np.int32)
    mask_c = np.repeat(swap_mask, 2, axis=1)  # [b, 32]
    perm = np.where(mask_c, cidx[None, :] ^ 1, cidx[None, :]).astype(np.int32)

    in_maps = []
    for m in range(M):
        xs = np.ascontiguousarray(X[m * BL : (m + 1) * BL])  # [BL, C, T]
        pm = perm[m * BL : (m + 1) * BL]  # [BL, 32]
        # flat source row for (local batch bl, out channel ch<32)
        rows = (np.arange(BL, dtype=np.int32)[:, None] * c + pm).reshape(-1)
        idxm = np.ascontiguousarray(rows.reshape(-1, P).T.astype(np.int32))
        in_maps.append({"x": xs, "idx": idxm})
    return in_maps


def make_in_maps_v4(X, swap_mask):
    X = np.asarray(X, dtype=np.float32)
    swap_mask = np.asarray(swap_mask).astype(bool)
    b, c, t = X.shape
    half = c // 2

    cidx = np.arange(half, dtype=np.int32)
    mask_c = np.repeat(swap_mask, 2, axis=1)
    perm = np.where(mask_c, cidx[None, :] ^ 1, cidx[None, :]).astype(np.int32)

    nchunk = BL * half // P
    bpc = P // half
    in_maps, init_outs = [], []
    for m in range(M):
        pm = perm[m * BL : (m + 1) * BL]
        rows = (np.arange(BL, dtype=np.int32)[:, None] * c + pm).reshape(-1)
        idxm = np.ascontiguousarray(rows.reshape(-1, P).T.astype(np.int32))
        in_maps.append({"idx": idxm})
        init_outs.append({"y": np.ascontiguousarray(X[m * BL : (m + 1) * BL])})
    return in_maps, init_outs


class _V4Result:
    def __init__(self, exec_time_ns=None):
        self.exec_time_ns = exec_time_ns
        self.mean_exec_time_ns = exec_time_ns


def _ntff_capture(output_dir, device_ids):
    """Self-contained NTFF capture via libaxon_pjrt.so (trace path only)."""
    import contextlib as _cl
    import ctypes

    lib = ctypes.CDLL("/opt/axon/libaxon_pjrt.so")
    lib.axon_start_nrt_profile.argtypes = [
        ctypes.POINTER(ctypes.c_int64),
        ctypes.c_size_t,
    ]
    lib.axon_start_nrt_profile.restype = ctypes.c_int64
    lib.axon_stop_nrt_profile.argtypes = [ctypes.c_char_p]
    lib.axon_stop_nrt_profile.restype = ctypes.c_int64

    @_cl.contextmanager
    def _hook():
        import jax

        jax.devices()
        ids = (ctypes.c_int64 * len(device_ids))(*device_ids)
        rc = lib.axon_start_nrt_profile(ids, len(device_ids))
        if rc != 0:
            raise RuntimeError(f"axon_start_nrt_profile rc={rc}")
        try:
            yield
        finally:
            n = lib.axon_stop_nrt_profile(str(output_dir).encode())
            print(f"profile: {n} file(s) in {output_dir}", file=sys.stderr)

    return _hook()


SPLIT = 4


def _run_v4(X, swap_mask, trace=False):
    if VERSION == 9:
        in_maps, init_outs, nchunk = make_in_maps_v9(X, swap_mask, SPLIT)
        nc = build_bass_v9(nchunk, nbuf=min(nchunk, 6), split=SPLIT)
    elif VERSION == 8:
        in_maps, init_outs, nchunk = make_in_maps_v8(X, swap_mask, SPLIT)
        nc = build_bass_v8(nchunk, nbuf=min(nchunk, 6), split=SPLIT)
    elif VERSION == 7:
        in_maps, init_outs, nchunk = make_in_maps_v7(X, swap_mask)
        nc = build_bass_v7(nchunk, nbuf=min(nchunk, 6))
    elif VERSION == 6:
        nc = build_bass_v6()
        in_maps, init_outs = make_in_maps_v6(X, swap_mask)
    else:
        nc = build_bass_v5() if VERSION == 5 else build_bass_v4()
        in_maps, init_outs = make_in_maps_v4(X, swap_mask)
    nc.finalize()
    exec_time_ns = None
    if trace:
        import glob
        import os
        import tempfile

        neff_dir = tempfile.mkdtemp()
        with _ntff_capture(neff_dir, [0]):
            results = _run_pjrt_with_init(nc, in_maps, init_outs)
        ntffs = glob.glob(os.path.join(neff_dir, "*_body*.ntff"))
        if ntffs:
            import gauge.profiler
            from concourse.bass_utils import FishPath

            profile = gauge.profiler.Profile(
                profile_path=FishPath(neff_dir),
                kernel_dev_mode=True,
                profile_on_exit=False,
                bass_kernel=nc.m,
                offline_processing=True,
                fname="*_body*",
                metadata={"artifacts_path": f"local:{neff_dir}"},
            )
            pr = profile.to_perfetto(model_index=(0,))
            if pr:
                exec_time_ns = pr[0].exec_time_ns
            print(f"ntff json dir: {neff_dir}", file=sys.stderr)
    else:
        results = _run_pjrt_with_init(nc, in_maps, init_outs)
    out = np.concatenate([r["y"] for r in results], axis=0)
    return out, _V4Result(exec_time_ns)


VERSION = 7


def run(X, swap_mask, **kw):
    if VERSION in (4, 5, 6, 7, 8, 9):
        return _run_v4(X, swap_mask, trace=kw.get("trace", False))
    if VERSION == 2:
        nc = build_bass_v2()
        in_maps = make_in_maps_v2(X, swap_mask)
    else:
        nc = build_bass()
        in_maps = make_in_maps(X, swap_mask)
    if not nc.is_finalized():
        nc.finalize()
    res = run_bass_kernel_spmd(nc, in_maps, list(range(M)), **kw)
    out = np.concatenate(
        [r["y"].reshape(BL, C, T) for r in res.results], axis=0
    )
    return out, res


def kernel(X, swap_mask):
    out, _ = run(X, swap_mask)
    return out



# revision 18
# speedup vs baseline: 1.0764x; 1.0764x over previous
"""ChannelSymmetry kernel for Trainium2 (8 NeuronCores, SPMD data-parallel).

Problem: X [128, 64, 8000] f32, swap_mask [128, 16] bool. For each batch b and
channel pair p (channels 2p, 2p+1; p < 16, i.e. channels 0..31), swap the two
channel rows iff swap_mask[b, p]. Channels 32..63 pass through unchanged.

Design: the permutation is runtime data, so it cannot live in compile-time DMA
access patterns. The host turns swap_mask into per-row source indices; the
device does an indirect-DMA row gather (each row = 32KB contiguous, full DMA
efficiency) from HBM into SBUF, then a regular store back to HBM. Pure DMA,
no compute engines — this is a memory-roofline problem.

Sharding: pure data parallel over the batch axis, 16 batches per core.
"""

import contextlib
import sys

import numpy as np

for _p in ("/opt/trn_rl_repo", "/opt/pypackages"):
    if _p not in sys.path:
        sys.path.append(_p)

import concourse.bass as bass
import concourse.mybir as mybir
import concourse.tile as tile
from concourse.bass_utils import run_bass_kernel_spmd

B, C, T = 128, 64, 8000
M = 8            # cores
BL = B // M      # batches per core
ROWS = BL * C    # rows per core (viewing X_shard as [ROWS, T])
P = 128          # SBUF partitions / rows per chunk


def build_bass(rows=ROWS, t=T, nbuf=3):
    """Per-core program: for each chunk of 128 rows, indirect-gather the
    permuted source rows from HBM into SBUF, then store contiguously.

    Raw bass (no Tile): walrus only allows one sync-wait per DMA
    instruction, so waits must be standalone sequencer instructions.
    gpsimd (SWDGE) issues the gathers; sync (HWDGE) issues the stores;
    two semaphores ping-pong the nbuf SBUF slots between them.
    """
    nchunk = rows // P
    nc = bass.Bass()
    x = nc.dram_tensor("x", [rows, t], mybir.dt.float32, kind="ExternalInput")
    idx = nc.dram_tensor("idx", [P, nchunk], mybir.dt.int32, kind="ExternalInput")
    y = nc.dram_tensor("y", [rows, t], mybir.dt.float32, kind="ExternalOutput")

    with contextlib.ExitStack() as ctx:
        idx_t = ctx.enter_context(
            nc.sbuf_tensor("idx_t", [P, nchunk], mybir.dt.int32)
        )
        bufs = [
            ctx.enter_context(nc.sbuf_tensor(f"buf{i}", [P, t], mybir.dt.float32))
            for i in range(nbuf)
        ]
        i_sem = ctx.enter_context(nc.semaphore(name="i_sem"))
        g_sems = [
            ctx.enter_context(nc.semaphore(name=f"g_sem{i}")) for i in range(nbuf)
        ]
        s_sems = [
            ctx.enter_context(nc.semaphore(name=f"s_sem{i}")) for i in range(nbuf)
        ]
        block = ctx.enter_context(nc.Block())

        @block.gpsimd
        def _(g):
            g.dma_start(out=idx_t[:], in_=idx[:]).then_inc(i_sem, 16)
            g.wait_ge(i_sem, 16)
            for ci in range(nchunk):
                sl, rnd = ci % nbuf, ci // nbuf
                if rnd > 0:
                    # slot free once its previous store completed
                    g.wait_ge(s_sems[sl], rnd * 16)
                g.indirect_dma_start(
                    out=bufs[sl][:],
                    out_offset=None,
                    in_=x[:],
                    in_offset=bass.IndirectOffsetOnAxis(
                        ap=idx_t[:, ci : ci + 1], axis=0
                    ),
                ).then_inc(g_sems[sl], 16)

        @block.sync
        def _(s):
            for ci in range(nchunk):
                sl, rnd = ci % nbuf, ci // nbuf
                s.wait_ge(g_sems[sl], (rnd + 1) * 16)
                s.dma_start(
                    out=y[ci * P : (ci + 1) * P, :], in_=bufs[sl][:]
                ).then_inc(s_sems[sl], 16)
            # drain: every slot's stores complete before kernel end
            for sl in range(nbuf):
                nstores = (nchunk - sl + nbuf - 1) // nbuf
                if nstores > 0:
                    s.wait_ge(s_sems[sl], nstores * 16)

    return nc


def build_bass_v2(bl=BL, c=C, t=T, nbuf=3):
    """v2: only the 32 swappable channels go through the SBUF gather+store
    path; the 32 pass-through channels move as direct DRAM->DRAM copies on
    the ACT HWDGE ring. Stream traffic drops from 2x to 1.5x of data size
    and spreads evenly over the three DMA rings (Pool/SP/ACT).
    """
    assert c == 64
    half = c // 2
    rows = bl * c
    grows = bl * half          # gathered rows (channels 0..31 of each batch)
    nchunk = grows // P        # 4 batches per chunk
    assert grows % P == 0
    bpc = P // half            # batches per gather chunk (=4)
    nc = bass.Bass()
    x = nc.dram_tensor("x", [bl, c, t], mybir.dt.float32, kind="ExternalInput")
    idx = nc.dram_tensor("idx", [P, nchunk], mybir.dt.int32, kind="ExternalInput")
    y = nc.dram_tensor("y", [bl, c, t], mybir.dt.float32, kind="ExternalOutput")
    x_flat = x.rearrange("b c t -> (b c) t")

    with contextlib.ExitStack() as ctx:
        idx_t = ctx.enter_context(
            nc.sbuf_tensor("idx_t", [P, nchunk], mybir.dt.int32)
        )
        bufs = [
            ctx.enter_context(nc.sbuf_tensor(f"buf{i}", [P, t], mybir.dt.float32))
            for i in range(nbuf)
        ]
        i_sem = ctx.enter_context(nc.semaphore(name="i_sem"))
        g_sems = [
            ctx.enter_context(nc.semaphore(name=f"g_sem{i}")) for i in range(nbuf)
        ]
        s_sems = [
            ctx.enter_context(nc.semaphore(name=f"s_sem{i}")) for i in range(nbuf)
        ]
        d_sem = ctx.enter_context(nc.semaphore(name="d_sem"))
        block = ctx.enter_context(nc.Block())

        @block.scalar
        def _(a):
            # independent pass-through copies, one per gather-chunk's batches
            for ci in range(nchunk):
                a.dma_start(
                    out=y[ci * bpc : (ci + 1) * bpc, half:c, :],
                    in_=x[ci * bpc : (ci + 1) * bpc, half:c, :],
                ).then_inc(d_sem, 16)
            a.wait_ge(d_sem, nchunk * 16)

        @block.gpsimd
        def _(g):
            g.dma_start(out=idx_t[:], in_=idx[:]).then_inc(i_sem, 16)
            g.wait_ge(i_sem, 16)
            for ci in range(nchunk):
                sl, rnd = ci % nbuf, ci // nbuf
                if rnd > 0:
                    g.wait_ge(s_sems[sl], rnd * 16)
                g.indirect_dma_start(
                    out=bufs[sl][:],
                    out_offset=None,
                    in_=x_flat[:],
                    in_offset=bass.IndirectOffsetOnAxis(
                        ap=idx_t[:, ci : ci + 1], axis=0
                    ),
                ).then_inc(g_sems[sl], 16)

        @block.sync
        def _(s):
            for ci in range(nchunk):
                sl, rnd = ci % nbuf, ci // nbuf
                s.wait_ge(g_sems[sl], (rnd + 1) * 16)
                s.dma_start(
                    out=y[ci * bpc : (ci + 1) * bpc, 0:half, :], in_=bufs[sl][:]
                ).then_inc(s_sems[sl], 16)
            for sl in range(nbuf):
                nstores = (nchunk - sl + nbuf - 1) // nbuf
                if nstores > 0:
                    s.wait_ge(s_sems[sl], nstores * 16)

    return nc


def build_bass_v4(bl=BL, c=C, t=T, nbuf=3):
    """v4: true in-place. `y` arrives pre-initialized with this core's X
    shard (donated PJRT buffer). Only channels 0..31 move: indirect-gather
    the permuted rows out of y itself into SBUF, then store them back.
    Channels 32..63 are never touched. Per-chunk pipelining is safe: chunk
    ci's gather reads exactly the rows chunk ci's store later writes, and
    different chunks touch disjoint row sets.
    """
    assert c == 64
    half = c // 2
    nchunk = bl * half // P    # gather chunks (4 batches each)
    bpc = P // half
    nc = bass.Bass()
    idx = nc.dram_tensor("idx", [P, nchunk], mybir.dt.int32, kind="ExternalInput")
    y = nc.dram_tensor("y", [bl, c, t], mybir.dt.float32, kind="ExternalOutput")
    y_flat = y.rearrange("b c t -> (b c) t")

    with contextlib.ExitStack() as ctx:
        idx_t = ctx.enter_context(
            nc.sbuf_tensor("idx_t", [P, nchunk], mybir.dt.int32)
        )
        bufs = [
            ctx.enter_context(nc.sbuf_tensor(f"buf{i}", [P, t], mybir.dt.float32))
            for i in range(nbuf)
        ]
        i_sem = ctx.enter_context(nc.semaphore(name="i_sem"))
        g_sems = [
            ctx.enter_context(nc.semaphore(name=f"g_sem{i}")) for i in range(nbuf)
        ]
        s_sems = [
            ctx.enter_context(nc.semaphore(name=f"s_sem{i}")) for i in range(nbuf)
        ]
        block = ctx.enter_context(nc.Block())

        @block.gpsimd
        def _(g):
            g.dma_start(out=idx_t[:], in_=idx[:]).then_inc(i_sem, 16)
            g.wait_ge(i_sem, 16)
            for ci in range(nchunk):
                sl, rnd = ci % nbuf, ci // nbuf
                if rnd > 0:
                    g.wait_ge(s_sems[sl], rnd * 16)
                g.indirect_dma_start(
                    out=bufs[sl][:],
                    out_offset=None,
                    in_=y_flat[:],
                    in_offset=bass.IndirectOffsetOnAxis(
                        ap=idx_t[:, ci : ci + 1], axis=0
                    ),
                ).then_inc(g_sems[sl], 16)

        @block.sync
        def _(s):
            for ci in range(nchunk):
                sl, rnd = ci % nbuf, ci // nbuf
                s.wait_ge(g_sems[sl], (rnd + 1) * 16)
                s.dma_start(
                    out=y[ci * bpc : (ci + 1) * bpc, 0:half, :], in_=bufs[sl][:]
                ).then_inc(s_sems[sl], 16)
            for sl in range(nbuf):
                nstores = (nchunk - sl + nbuf - 1) // nbuf
                if nstores > 0:
                    s.wait_ge(s_sems[sl], nstores * 16)

    return nc


def build_bass_v5(bl=BL, c=C, t=T, nbuf=3):
    """v5: in-place like v4, but every DRAM-side AP is 2D contiguous
    (3D strided DRAM APs measured ~4.5x slower on HWDGE). Each gather
    chunk's 4 batches are stored as 4 separate 1MB contiguous stores.
    idx loads via HWDGE (sync) to shave SWDGE startup.
    """
    assert c == 64
    half = c // 2
    nchunk = bl * half // P    # 4 chunks of 4 batches
    bpc = P // half            # batches per chunk
    nc = bass.Bass()
    idx = nc.dram_tensor("idx", [P, nchunk], mybir.dt.int32, kind="ExternalInput")
    y = nc.dram_tensor("y", [bl, c, t], mybir.dt.float32, kind="ExternalOutput")
    y_flat = y.rearrange("b c t -> (b c) t")

    with contextlib.ExitStack() as ctx:
        idx_t = ctx.enter_context(
            nc.sbuf_tensor("idx_t", [P, nchunk], mybir.dt.int32)
        )
        bufs = [
            ctx.enter_context(nc.sbuf_tensor(f"buf{i}", [P, t], mybir.dt.float32))
            for i in range(nbuf)
        ]
        i_sem = ctx.enter_context(nc.semaphore(name="i_sem"))
        g_sems = [
            ctx.enter_context(nc.semaphore(name=f"g_sem{i}")) for i in range(nbuf)
        ]
        s_sems = [
            ctx.enter_context(nc.semaphore(name=f"s_sem{i}")) for i in range(nbuf)
        ]
        block = ctx.enter_context(nc.Block())

        @block.gpsimd
        def _(g):
            g.wait_ge(i_sem, 16)
            for ci in range(nchunk):
                sl, rnd = ci % nbuf, ci // nbuf
                if rnd > 0:
                    # slot free once its previous 4 stores completed
                    g.wait_ge(s_sems[sl], rnd * 64)
                g.indirect_dma_start(
                    out=bufs[sl][:],
                    out_offset=None,
                    in_=y_flat[:],
                    in_offset=bass.IndirectOffsetOnAxis(
                        ap=idx_t[:, ci : ci + 1], axis=0
                    ),
                ).then_inc(g_sems[sl], 16)

        @block.sync
        def _(s):
            s.dma_start(out=idx_t[:], in_=idx[:]).then_inc(i_sem, 16)
            for ci in range(nchunk):
                sl, rnd = ci % nbuf, ci // nbuf
                s.wait_ge(g_sems[sl], (rnd + 1) * 16)
                for j in range(bpc):
                    row0 = (ci * bpc + j) * c
                    s.dma_start(
                        out=y_flat[row0 : row0 + half, :],
                        in_=bufs[sl][j * half : (j + 1) * half, :],
                    ).then_inc(s_sems[sl], 16)
            for sl in range(nbuf):
                nstores = (nchunk - sl + nbuf - 1) // nbuf
                if nstores > 0:
                    s.wait_ge(s_sems[sl], nstores * 64)

    return nc


def build_bass_v6(bl=BL, c=C, t=T, nbuf=3):
    """v6: in-place + dma_gather (TIE-accelerated descriptor gen, ~0.34ns/desc
    vs ~127ns for indirect_dma_start) + stride-4 partition interleave so each
    batch's 1MB contiguous store spans all 16 SDMA engines.

    Gather position i of chunk ci = (batch i%4, channel i//4), so store j
    reads SBUF partitions j::4 and writes one contiguous 32-row block.
    """
    assert c == 64
    half = c // 2
    nchunk = bl * half // P
    bpc = P // half
    nc = bass.Bass()
    idx = nc.dram_tensor(
        "idx", [P, nchunk * 8], mybir.dt.int16, kind="ExternalInput"
    )
    y = nc.dram_tensor("y", [bl, c, t], mybir.dt.float32, kind="ExternalOutput")
    y_flat = y.rearrange("b c t -> (b c) t")

    with contextlib.ExitStack() as ctx:
        idx_t = ctx.enter_context(
            nc.sbuf_tensor("idx_t", [P, nchunk * 8], mybir.dt.int16)
        )
        bufs = [
            ctx.enter_context(
                nc.sbuf_tensor(f"buf{i}", [P, 1, t], mybir.dt.float32)
            )
            for i in range(nbuf)
        ]
        i_sem = ctx.enter_context(nc.semaphore(name="i_sem"))
        g_sems = [
            ctx.enter_context(nc.semaphore(name=f"g_sem{i}")) for i in range(nbuf)
        ]
        s_sems = [
            ctx.enter_context(nc.semaphore(name=f"s_sem{i}")) for i in range(nbuf)
        ]
        block = ctx.enter_context(nc.Block())

        @block.gpsimd
        def _(g):
            from concourse import library_config

            g.load_library(library_config.attnmlp)
            g.wait_ge(i_sem, 16)
            for ci in range(nchunk):
                sl, rnd = ci % nbuf, ci // nbuf
                if rnd > 0:
                    g.wait_ge(s_sems[sl], rnd * 64)
                g.dma_gather(
                    bufs[sl][:],
                    y_flat[:],
                    idx_t[:, ci * 8 : (ci + 1) * 8],
                    P,
                    P,
                    t,
                ).then_inc(g_sems[sl], 16)

        @block.sync
        def _(s):
            s.dma_start(out=idx_t[:], in_=idx[:]).then_inc(i_sem, 16)
            for ci in range(nchunk):
                sl, rnd = ci % nbuf, ci // nbuf
                s.wait_ge(g_sems[sl], (rnd + 1) * 16)
                for j in range(bpc):
                    row0 = (ci * bpc + j) * c
                    s.dma_start(
                        out=y_flat[row0 : row0 + half, :],
                        in_=bufs[sl][j : P : bpc, 0, :],
                    ).then_inc(s_sems[sl], 16)
            for sl in range(nbuf):
                nstores = (nchunk - sl + nbuf - 1) // nbuf
                if nstores > 0:
                    s.wait_ge(s_sems[sl], nstores * 64)

    return nc


def build_bass_v7(nchunk, nbuf, bl=BL, c=C, t=T):
    """v7: in-place, minimal traffic. Only rows whose pair actually swaps
    move: indirect-gather each swapped row's partner into SBUF, then
    indirect-scatter it back to the swapped row's slot. Cores with fewer
    swaps than the SPMD-wide max pad their index columns with OOB entries
    (idx > bounds_check, oob_is_err=False) which generate no descriptors.

    idx layout: [128, 2*nchunk] int32; col 2ci = gather (partner) rows,
    col 2ci+1 = scatter (destination) rows for chunk ci. Both rows of a
    pair sit in the same chunk, so pipelined chunks touch disjoint rows.
    """
    rows = bl * c
    nc = bass.Bass()
    idx = nc.dram_tensor(
        "idx", [P, 2 * nchunk], mybir.dt.int32, kind="ExternalInput"
    )
    y = nc.dram_tensor("y", [bl, c, t], mybir.dt.float32, kind="ExternalOutput")
    y_flat = y.rearrange("b c t -> (b c) t")

    with contextlib.ExitStack() as ctx:
        idx_t = ctx.enter_context(
            nc.sbuf_tensor("idx_t", [P, 2 * nchunk], mybir.dt.int32)
        )
        bufs = [
            ctx.enter_context(nc.sbuf_tensor(f"buf{i}", [P, t], mybir.dt.float32))
            for i in range(nbuf)
        ]
        i_sem = ctx.enter_context(nc.semaphore(name="i_sem"))
        g_sems = [
            ctx.enter_context(nc.semaphore(name=f"g_sem{i}")) for i in range(nbuf)
        ]
        s_sems = [
            ctx.enter_context(nc.semaphore(name=f"s_sem{i}")) for i in range(nbuf)
        ]
        block = ctx.enter_context(nc.Block())

        @block.gpsimd
        def _(g):
            def gather(ci):
                sl = ci % nbuf
                g.indirect_dma_start(
                    out=bufs[sl][:],
                    out_offset=None,
                    in_=y_flat[:],
                    in_offset=bass.IndirectOffsetOnAxis(
                        ap=idx_t[:, 2 * ci : 2 * ci + 1], axis=0
                    ),
                    bounds_check=rows - 1,
                    oob_is_err=False,
                ).then_inc(g_sems[sl], 16)

            def scatter(ci):
                sl = ci % nbuf
                g.wait_ge(g_sems[sl], (ci // nbuf + 1) * 16)
                g.indirect_dma_start(
                    out=y_flat[:],
                    out_offset=bass.IndirectOffsetOnAxis(
                        ap=idx_t[:, 2 * ci + 1 : 2 * ci + 2], axis=0
                    ),
                    in_=bufs[sl][:],
                    in_offset=None,
                    bounds_check=rows - 1,
                    oob_is_err=False,
                ).then_inc(s_sems[sl], 16)

            g.wait_ge(i_sem, 16)
            # software-pipelined: gathers run nbuf-1 chunks ahead of scatters
            for ci in range(nchunk):
                if ci >= nbuf:
                    g.wait_ge(s_sems[ci % nbuf], (ci // nbuf) * 16)
                gather(ci)
                cj = ci - (nbuf - 1)
                if cj >= 0:
                    scatter(cj)
            for cj in range(max(0, nchunk - (nbuf - 1)), nchunk):
                scatter(cj)
            for sl in range(nbuf):
                nst = (nchunk - sl + nbuf - 1) // nbuf
                if nst > 0:
                    g.wait_ge(s_sems[sl], nst * 16)

        @block.sync
        def _(s):
            s.dma_start(out=idx_t[:], in_=idx[:]).then_inc(i_sem, 16)

    return nc


def build_bass_v8(nchunk, nbuf, split, bl=BL, c=C, t=T):
    """v8: v7 with each 32KB row split into `split` sub-row descriptors.
    The SWDGE deals descriptors to the 16 SDMA engines in blocks of 8, so
    smaller descriptors shrink the per-engine granularity (load imbalance
    from partial tail chunks drops from ~10us to ~10/split us).

    idx layout: [128, 2*split*nchunk] int32 into y viewed as
    [(b c split), t/split]. Chunk ci: cols [2s*ci, 2s*ci+s) = gather descs
    (desc j of the chunk feeds buf partition j//s, sub-row j%s), cols
    [2s*ci+s, 2s*ci+2s) = scatter descs.
    """
    s_ = split
    rows = bl * c * s_
    ts = t // s_
    nc = bass.Bass()
    idx = nc.dram_tensor(
        "idx", [P, 2 * s_ * nchunk], mybir.dt.int32, kind="ExternalInput"
    )
    y = nc.dram_tensor("y", [bl, c, t], mybir.dt.float32, kind="ExternalOutput")
    y_sub = y.rearrange("b c (s x) -> (b c s) x", s=s_)

    with contextlib.ExitStack() as ctx:
        idx_t = ctx.enter_context(
            nc.sbuf_tensor("idx_t", [P, 2 * s_ * nchunk], mybir.dt.int32)
        )
        bufs = [
            ctx.enter_context(nc.sbuf_tensor(f"buf{i}", [P, t], mybir.dt.float32))
            for i in range(nbuf)
        ]
        i_sem = ctx.enter_context(nc.semaphore(name="i_sem"))
        g_sems = [
            ctx.enter_context(nc.semaphore(name=f"g_sem{i}")) for i in range(nbuf)
        ]
        s_sems = [
            ctx.enter_context(nc.semaphore(name=f"s_sem{i}")) for i in range(nbuf)
        ]
        block = ctx.enter_context(nc.Block())

        @block.gpsimd
        def _(g):
            def gather(ci):
                sl = ci % nbuf
                a = 2 * s_ * ci
                g.indirect_dma_start(
                    out=bufs[sl][:],
                    out_offset=None,
                    in_=y_sub[:],
                    in_offset=bass.IndirectOffsetOnAxis(
                        ap=idx_t[:, a : a + s_], axis=0
                    ),
                    bounds_check=rows - 1,
                    oob_is_err=False,
                ).then_inc(g_sems[sl], 16)

            def scatter(ci):
                sl = ci % nbuf
                a = 2 * s_ * ci + s_
                g.wait_ge(g_sems[sl], (ci // nbuf + 1) * 16)
                g.indirect_dma_start(
                    out=y_sub[:],
                    out_offset=bass.IndirectOffsetOnAxis(
                        ap=idx_t[:, a : a + s_], axis=0
                    ),
                    in_=bufs[sl][:],
                    in_offset=None,
                    bounds_check=rows - 1,
                    oob_is_err=False,
                ).then_inc(s_sems[sl], 16)

            g.wait_ge(i_sem, 16)
            for ci in range(nchunk):
                if ci >= nbuf:
                    g.wait_ge(s_sems[ci % nbuf], (ci // nbuf) * 16)
                gather(ci)
                cj = ci - (nbuf - 1)
                if cj >= 0:
                    scatter(cj)
            for cj in range(max(0, nchunk - (nbuf - 1)), nchunk):
                scatter(cj)
            for sl in range(nbuf):
                nst = (nchunk - sl + nbuf - 1) // nbuf
                if nst > 0:
                    g.wait_ge(s_sems[sl], nst * 16)

        @block.sync
        def _(s):
            s.dma_start(out=idx_t[:], in_=idx[:]).then_inc(i_sem, 16)

    return nc


def build_bass_v11(caps, nbuf, bl=BL, c=C, t=T):
    """v11: full 128-position chunks plus one partial-AP tail chunk.
    caps[ci] = offset-AP position count of chunk ci (128 for full chunks;
    the tail's count is a multiple of 16 so the DGE's position-slice
    dealing spreads it across all 16 engines). Index columns hold OOB
    entries (skipped at descriptor gen) wherever a core has fewer swaps.
    """
    rows = bl * c
    nchunk = len(caps)
    nc = bass.Bass()
    idx = nc.dram_tensor(
        "idx", [P, 2 * nchunk], mybir.dt.int32, kind="ExternalInput"
    )
    y = nc.dram_tensor("y", [bl, c, t], mybir.dt.float32, kind="ExternalOutput")
    y_flat = y.rearrange("b c t -> (b c) t")

    with contextlib.ExitStack() as ctx:
        idx_t = ctx.enter_context(
            nc.sbuf_tensor("idx_t", [P, 2 * nchunk], mybir.dt.int32)
        )
        bufs = [
            ctx.enter_context(nc.sbuf_tensor(f"buf{i}", [P, t], mybir.dt.float32))
            for i in range(nbuf)
        ]
        i_sem = ctx.enter_context(nc.semaphore(name="i_sem"))
        g_sems = [
            ctx.enter_context(nc.semaphore(name=f"g_sem{i}")) for i in range(nbuf)
        ]
        s_sems = [
            ctx.enter_context(nc.semaphore(name=f"s_sem{i}")) for i in range(nbuf)
        ]
        block = ctx.enter_context(nc.Block())

        @block.gpsimd
        def _(g):
            def gather(ci):
                sl, np_ = ci % nbuf, caps[ci]
                g.indirect_dma_start(
                    out=bufs[sl][:np_, :],
                    out_offset=None,
                    in_=y_flat[:],
                    in_offset=bass.IndirectOffsetOnAxis(
                        ap=idx_t[:np_, 2 * ci : 2 * ci + 1], axis=0
                    ),
                    bounds_check=rows - 1,
                    oob_is_err=False,
                ).then_inc(g_sems[sl], 16)

            def scatter(ci):
                sl, np_ = ci % nbuf, caps[ci]
                g.wait_ge(g_sems[sl], (ci // nbuf + 1) * 16)
                g.indirect_dma_start(
                    out=y_flat[:],
                    out_offset=bass.IndirectOffsetOnAxis(
                        ap=idx_t[:np_, 2 * ci + 1 : 2 * ci + 2], axis=0
                    ),
                    in_=bufs[sl][:np_, :],
                    in_offset=None,
                    bounds_check=rows - 1,
                    oob_is_err=False,
                ).then_inc(s_sems[sl], 16)

            g.wait_ge(i_sem, 16)
            for ci in range(nchunk):
                if ci >= nbuf:
                    g.wait_ge(s_sems[ci % nbuf], (ci // nbuf) * 16)
                gather(ci)
                cj = ci - (nbuf - 1)
                if cj >= 0:
                    scatter(cj)
            for cj in range(max(0, nchunk - (nbuf - 1)), nchunk):
                scatter(cj)
            for sl in range(nbuf):
                nst = (nchunk - sl + nbuf - 1) // nbuf
                if nst > 0:
                    g.wait_ge(s_sems[sl], nst * 16)

        @block.sync
        def _(s):
            s.dma_start(out=idx_t[:], in_=idx[:]).then_inc(i_sem, 16)

    return nc


def make_in_maps_v11(X, swap_mask):
    """Balanced batch->core assignment (LPT on per-batch swap rows) plus
    per-chunk even spreading of real entries.

    Returns (in_maps, init_outs, caps, assign) where assign[m] lists the
    16 global batch ids owned by core m (output must be un-permuted)."""
    X = np.asarray(X, dtype=np.float32)
    swap_mask = np.asarray(swap_mask).astype(bool)
    b, c, t = X.shape

    # LPT: heaviest batches first onto the least-loaded core with room
    w = 2 * swap_mask.sum(axis=1)  # rows to move per batch
    order = np.argsort(-w, kind="stable")
    loads = [0] * M
    counts = [0] * M
    assign = [[] for _ in range(M)]
    for bi in order:
        m = min(
            (mm for mm in range(M) if counts[mm] < BL),
            key=lambda mm: (loads[mm], mm),
        )
        assign[m].append(int(bi))
        loads[m] += int(w[bi])
        counts[m] += 1

    src_lists, dst_lists = [], []
    for m in range(M):
        sm = swap_mask[assign[m]]  # [BL, 16] in local batch order
        blv, pv = np.nonzero(sm)
        a = (blv * c + 2 * pv).astype(np.int32)
        src = np.empty(2 * a.size, dtype=np.int32)
        dst = np.empty(2 * a.size, dtype=np.int32)
        src[0::2], src[1::2] = a + 1, a
        dst[0::2], dst[1::2] = a, a + 1
        src_lists.append(src)
        dst_lists.append(dst)

    lmax = max(s.size for s in src_lists)
    nfull = lmax // P
    tail = lmax - nfull * P
    caps = [P] * nfull + ([min(P, 16 * -(-tail // 16))] if tail else [])
    if not caps:
        caps = [16]

    in_maps, init_outs = [], []
    for m in range(M):
        srcl, dstl = src_lists[m], dst_lists[m]
        n = srcl.size
        idxm = np.full((P, 2 * len(caps)), OOB_PAD, dtype=np.int32)
        off = 0
        for ci, cap in enumerate(caps):
            take = min(cap, n - off)
            if take > 0:
                pos = (np.arange(take) * cap) // take
                idxm[pos, 2 * ci] = srcl[off : off + take]
                idxm[pos, 2 * ci + 1] = dstl[off : off + take]
            off += take
        in_maps.append({"idx": np.ascontiguousarray(idxm)})
        init_outs.append({"y": np.ascontiguousarray(X[assign[m]])})
    return in_maps, init_outs, caps, assign


def build_bass_v9(nchunk, nbuf, split, bl=BL, c=C, t=T):
    """v9: like v8 but each chunk/direction issues `split` sub-instructions;
    sub-instruction k moves only sub-row k of every row (128 descriptors of
    32000/split bytes, strided a full row apart, so the DGE coalescer cannot
    re-merge them). Engine-dealing quantum drops 8x32KB -> 8x(32KB/split).

    idx layout: [128, 2*split*nchunk]; col 2s*ci+k = gather sub-instr k of
    chunk ci (values src_row*split+k), col 2s*ci+s+k = scatter sub-instr k.
    """
    s_ = split
    rows = bl * c * s_
    ts = t // s_
    nc = bass.Bass()
    idx = nc.dram_tensor(
        "idx", [P, 2 * s_ * nchunk], mybir.dt.int32, kind="ExternalInput"
    )
    y = nc.dram_tensor("y", [bl, c, t], mybir.dt.float32, kind="ExternalOutput")
    y_sub = y.rearrange("b c (s x) -> (b c s) x", s=s_)

    with contextlib.ExitStack() as ctx:
        idx_t = ctx.enter_context(
            nc.sbuf_tensor("idx_t", [P, 2 * s_ * nchunk], mybir.dt.int32)
        )
        bufs = [
            ctx.enter_context(nc.sbuf_tensor(f"buf{i}", [P, t], mybir.dt.float32))
            for i in range(nbuf)
        ]
        i_sem = ctx.enter_context(nc.semaphore(name="i_sem"))
        g_sems = [
            ctx.enter_context(nc.semaphore(name=f"g_sem{i}")) for i in range(nbuf)
        ]
        s_sems = [
            ctx.enter_context(nc.semaphore(name=f"s_sem{i}")) for i in range(nbuf)
        ]
        block = ctx.enter_context(nc.Block())

        @block.gpsimd
        def _(g):
            def gather(ci):
                sl = ci % nbuf
                for k in range(s_):
                    a = 2 * s_ * ci + k
                    g.indirect_dma_start(
                        out=bufs[sl][:, k * ts : (k + 1) * ts],
                        out_offset=None,
                        in_=y_sub[:],
                        in_offset=bass.IndirectOffsetOnAxis(
                            ap=idx_t[:, a : a + 1], axis=0
                        ),
                        bounds_check=rows - 1,
                        oob_is_err=False,
                    ).then_inc(g_sems[sl], 16)

            def scatter(ci):
                sl = ci % nbuf
                g.wait_ge(g_sems[sl], (ci // nbuf + 1) * s_ * 16)
                for k in range(s_):
                    a = 2 * s_ * ci + s_ + k
                    g.indirect_dma_start(
                        out=y_sub[:],
                        out_offset=bass.IndirectOffsetOnAxis(
                            ap=idx_t[:, a : a + 1], axis=0
                        ),
                        in_=bufs[sl][:, k * ts : (k + 1) * ts],
                        in_offset=None,
                        bounds_check=rows - 1,
                        oob_is_err=False,
                    ).then_inc(s_sems[sl], 16)

            g.wait_ge(i_sem, 16)
            for ci in range(nchunk):
                if ci >= nbuf:
                    g.wait_ge(s_sems[ci % nbuf], (ci // nbuf) * s_ * 16)
                gather(ci)
                cj = ci - (nbuf - 1)
                if cj >= 0:
                    scatter(cj)
            for cj in range(max(0, nchunk - (nbuf - 1)), nchunk):
                scatter(cj)
            for sl in range(nbuf):
                nst = (nchunk - sl + nbuf - 1) // nbuf
                if nst > 0:
                    g.wait_ge(s_sems[sl], nst * s_ * 16)

        @block.sync
        def _(s):
            s.dma_start(out=idx_t[:], in_=idx[:]).then_inc(i_sem, 16)

    return nc


def make_in_maps_v9(X, swap_mask, split):
    """Row lists as v7; idx col (2s*ci + dir*s + k) = chunk ci's row
    indices *split + k (identity slot mapping, sub-row k per column)."""
    X = np.asarray(X, dtype=np.float32)
    swap_mask = np.asarray(swap_mask).astype(bool)
    b, c, t = X.shape

    src_lists, dst_lists = [], []
    for m in range(M):
        sm = swap_mask[m * BL : (m + 1) * BL]
        blv, pv = np.nonzero(sm)
        a = (blv * c + 2 * pv).astype(np.int32)
        src = np.empty(2 * a.size, dtype=np.int32)
        dst = np.empty(2 * a.size, dtype=np.int32)
        src[0::2], src[1::2] = a + 1, a
        dst[0::2], dst[1::2] = a, a + 1
        src_lists.append(src)
        dst_lists.append(dst)

    lmax = max(s.size for s in src_lists)
    nchunk = max(1, -(-lmax // P))
    lpad = nchunk * P

    in_maps, init_outs = [], []
    for m in range(M):
        src = np.full(lpad, OOB_PAD, dtype=np.int32)
        dst = np.full(lpad, OOB_PAD, dtype=np.int32)
        src[: src_lists[m].size] = src_lists[m]
        dst[: dst_lists[m].size] = dst_lists[m]
        srcc = src.reshape(nchunk, P)
        dstc = dst.reshape(nchunk, P)
        idxm = np.empty((P, 2 * split * nchunk), dtype=np.int32)
        for ci in range(nchunk):
            for k in range(split):
                idxm[:, 2 * split * ci + k] = srcc[ci] * split + k
                idxm[:, 2 * split * ci + split + k] = dstc[ci] * split + k
        in_maps.append({"idx": np.ascontiguousarray(idxm)})
        init_outs.append({"y": np.ascontiguousarray(X[m * BL : (m + 1) * BL])})
    return in_maps, init_outs, nchunk


def make_in_maps_v8(X, swap_mask, split):
    """Like v7 but indices address sub-rows (row r -> split descs
    r*split+q), interleaved per chunk as [gather s cols][scatter s cols]."""
    X = np.asarray(X, dtype=np.float32)
    swap_mask = np.asarray(swap_mask).astype(bool)
    b, c, t = X.shape

    src_lists, dst_lists = [], []
    for m in range(M):
        sm = swap_mask[m * BL : (m + 1) * BL]
        blv, pv = np.nonzero(sm)
        a = (blv * c + 2 * pv).astype(np.int32)
        src = np.empty(2 * a.size, dtype=np.int32)
        dst = np.empty(2 * a.size, dtype=np.int32)
        src[0::2], src[1::2] = a + 1, a
        dst[0::2], dst[1::2] = a, a + 1
        src_lists.append(src)
        dst_lists.append(dst)

    lmax = max(s.size for s in src_lists)
    nchunk = max(1, -(-lmax // P))
    lpad = nchunk * P

    in_maps, init_outs = [], []
    qoff = np.arange(split, dtype=np.int32)
    for m in range(M):
        src = np.full(lpad, OOB_PAD, dtype=np.int32)
        dst = np.full(lpad, OOB_PAD, dtype=np.int32)
        src[: src_lists[m].size] = src_lists[m]
        dst[: dst_lists[m].size] = dst_lists[m]
        # sub-row descs: [lpad, split]; OOB rows stay OOB (pad*split+q > bound)
        srcq = src[:, None] * split + qoff[None, :]
        dstq = dst[:, None] * split + qoff[None, :]
        # -> [nchunk, P, split] -> idx[p, 2s*ci + q] etc.
        idxm = np.empty((P, 2 * split * nchunk), dtype=np.int32)
        srcq = srcq.reshape(nchunk, P, split)
        dstq = dstq.reshape(nchunk, P, split)
        # slot shuffle: buf slot (p, q) <- entry (p+q)%P, quarter q, so
        # consecutive descriptors touch different DRAM rows and the DGE
        # cannot re-aggregate them into 32KB descriptors
        pidx = (np.arange(P)[:, None] + qoff[None, :]) % P  # [P, split]
        srcq = srcq[:, pidx, qoff[None, :]]
        dstq = dstq[:, pidx, qoff[None, :]]
        for ci in range(nchunk):
            idxm[:, 2 * split * ci : 2 * split * ci + split] = srcq[ci]
            idxm[:, 2 * split * ci + split : 2 * split * (ci + 1)] = dstq[ci]
        in_maps.append({"idx": np.ascontiguousarray(idxm)})
        init_outs.append({"y": np.ascontiguousarray(X[m * BL : (m + 1) * BL])})
    return in_maps, init_outs, nchunk


OOB_PAD = 1 << 20


def make_in_maps_v7(X, swap_mask):
    """Per-core (src, dst) row lists for swapped pairs only, padded with
    OOB entries to the max core's length rounded up to full 128-chunks."""
    X = np.asarray(X, dtype=np.float32)
    swap_mask = np.asarray(swap_mask).astype(bool)
    b, c, t = X.shape

    src_lists, dst_lists = [], []
    for m in range(M):
        sm = swap_mask[m * BL : (m + 1) * BL]  # [BL, 16]
        blv, pv = np.nonzero(sm)
        a = (blv * c + 2 * pv).astype(np.int32)  # even row of each pair
        # entries appended in pair order: (dst=a, src=a+1), (dst=a+1, src=a)
        src = np.empty(2 * a.size, dtype=np.int32)
        dst = np.empty(2 * a.size, dtype=np.int32)
        src[0::2], src[1::2] = a + 1, a
        dst[0::2], dst[1::2] = a, a + 1
        src_lists.append(src)
        dst_lists.append(dst)

    lmax = max(s.size for s in src_lists)
    nchunk = max(1, -(-lmax // P))
    lpad = nchunk * P

    in_maps, init_outs = [], []
    for m in range(M):
        src = np.full(lpad, OOB_PAD, dtype=np.int32)
        dst = np.full(lpad, OOB_PAD, dtype=np.int32)
        n = src_lists[m].size
        nfull = (n // P) * P
        src[:nfull] = src_lists[m][:nfull]
        dst[:nfull] = dst_lists[m][:nfull]
        rem = n - nfull
        if rem:
            # The DGE deals each instruction's descriptor list to the 16
            # engines as equal contiguous position slices (pre-OOB-skip,
            # slice->engine mapping is some fixed permutation). Round the
            # partial chunk's real count up to a multiple of 16 with
            # harmless self-copy entries (rows >= ch32 never swap), then
            # place them at a stride dividing 8 so every slice gets an
            # equal share no matter how slices map to engines.
            remp = min(P, 16 * -(-rem // 16))
            npad = remp - rem
            tail_src = np.concatenate(
                [src_lists[m][nfull:], 32 + np.arange(npad, dtype=np.int32)]
            )
            tail_dst = np.concatenate(
                [dst_lists[m][nfull:], 32 + np.arange(npad, dtype=np.int32)]
            )
            pos = nfull + (np.arange(remp) * P // remp)
            src[pos] = tail_src
            dst[pos] = tail_dst
        # idx[p, 2*ci] = src of entry ci*P+p; idx[p, 2*ci+1] = dst
        idxm = np.empty((P, 2 * nchunk), dtype=np.int32)
        idxm[:, 0::2] = src.reshape(nchunk, P).T
        idxm[:, 1::2] = dst.reshape(nchunk, P).T
        in_maps.append({"idx": np.ascontiguousarray(idxm)})
        init_outs.append({"y": np.ascontiguousarray(X[m * BL : (m + 1) * BL])})
    return in_maps, init_outs, nchunk


def make_in_maps_v6(X, swap_mask):
    X = np.asarray(X, dtype=np.float32)
    swap_mask = np.asarray(swap_mask).astype(bool)
    b, c, t = X.shape
    half = c // 2
    nchunk = BL * half // P
    bpc = P // half

    cidx = np.arange(half, dtype=np.int32)
    mask_c = np.repeat(swap_mask, 2, axis=1)
    perm = np.where(mask_c, cidx[None, :] ^ 1, cidx[None, :]).astype(np.int32)

    in_maps, init_outs = [], []
    for m in range(M):
        pm = perm[m * BL : (m + 1) * BL]  # [BL, 32]
        idx16 = np.zeros((P, nchunk * 8), dtype=np.int16)
        for ci in range(nchunk):
            for i in range(P):
                j, k = i % bpc, i // bpc
                bl_loc = ci * bpc + j
                idx16[i % 16, ci * 8 + i // 16] = bl_loc * c + pm[bl_loc, k]
        in_maps.append({"idx": idx16})
        init_outs.append({"y": np.ascontiguousarray(X[m * BL : (m + 1) * BL])})
    return in_maps, init_outs


def _run_pjrt_with_init(nc, in_maps, init_out_maps, n_cores=M):
    """Execute `nc` via PJRT on n_cores devices, donating PRE-INITIALIZED
    output buffers (instead of bass2jax's zeros) so in-place kernels see
    their starting contents. Mirrors concourse.bass2jax.run_bass_via_pjrt.
    """
    import jax
    from jax.experimental.shard_map import shard_map
    from jax.sharding import Mesh, PartitionSpec

    from concourse import bass2jax as b2j

    b2j.install_neuronx_cc_hook()
    assert nc.dbg_addr is None
    partition_name = (
        nc.partition_id_tensor.name if nc.partition_id_tensor else None
    )

    in_names, out_names, out_avals, out_shapes = [], [], [], []
    for alloc in nc.m.functions[0].allocations:
        if not isinstance(alloc, mybir.MemoryLocationSet):
            continue
        name = alloc.memorylocations[0].name
        if alloc.kind == "ExternalInput":
            if name != partition_name:
                in_names.append(name)
        elif alloc.kind == "ExternalOutput":
            shape = tuple(alloc.tensor_shape)
            dtype = mybir.dt.np(alloc.dtype)
            out_names.append(name)
            out_shapes.append((shape, dtype))
            out_avals.append(jax.core.ShapedArray(shape, dtype))
    n_params = len(in_names)
    n_outs = len(out_names)
    all_in_names = list(in_names) + list(out_names)
    if partition_name is not None:
        all_in_names.append(partition_name)

    donate = tuple(range(n_params, n_params + n_outs))

    def _body(*args):
        operands = list(args)
        if partition_name is not None:
            operands.append(b2j.partition_id_tensor())
        outs = b2j._bass_exec_p.bind(
            *operands,
            out_avals=tuple(out_avals),
            in_names=tuple(all_in_names),
            out_names=tuple(out_names),
            lowering_input_output_aliases=(),
            sim_require_finite=True,
            sim_require_nnan=True,
            nc=nc,
        )
        return tuple(outs)

    devices = jax.devices()[:n_cores]
    assert len(devices) == n_cores
    mesh = Mesh(np.asarray(devices), ("core",))
    in_specs = (PartitionSpec("core"),) * (n_params + n_outs)
    out_specs = (PartitionSpec("core"),) * n_outs
    sharded = jax.jit(
        shard_map(
            _body, mesh=mesh, in_specs=in_specs, out_specs=out_specs,
            check_rep=False,
        ),
        donate_argnums=donate,
        keep_unused=True,
    )
    concat_in = [
        np.concatenate(
            [np.asarray(m[name]) for m in in_maps], axis=0
        )
        for name in in_names
    ]
    concat_init = [
        np.concatenate(
            [np.asarray(m[name]) for m in init_out_maps], axis=0
        )
        for name in out_names
    ]
    out_arrs = sharded(*concat_in, *concat_init)
    return [
        {
            name: np.asarray(out_arrs[i]).reshape(
                n_cores, *out_shapes[i][0]
            )[ci]
            for i, name in enumerate(out_names)
        }
        for ci in range(n_cores)
    ]


def make_in_maps(X, swap_mask):
    X = np.asarray(X, dtype=np.float32)
    swap_mask = np.asarray(swap_mask).astype(bool)
    b, c, t = X.shape

    # Source-channel permutation per batch: perm[b, ch] = channel to read.
    cidx = np.arange(c, dtype=np.int32)
    partner = np.where(cidx < 32, cidx ^ 1, cidx).astype(np.int32)
    mask_c = np.zeros((b, c), dtype=bool)
    mask_c[:, :32] = np.repeat(swap_mask, 2, axis=1)
    perm = np.where(mask_c, partner[None, :], cidx[None, :]).astype(np.int32)

    in_maps = []
    for m in range(M):
        xs = np.ascontiguousarray(X[m * BL : (m + 1) * BL].reshape(BL * c, t))
        pm = perm[m * BL : (m + 1) * BL]  # [BL, c]
        rows = (np.arange(BL, dtype=np.int32)[:, None] * c + pm).reshape(-1)
        # idx[p, chunk] = source row feeding output row chunk*P + p
        idxm = np.ascontiguousarray(rows.reshape(-1, P).T.astype(np.int32))
        in_maps.append({"x": xs, "idx": idxm})
    return in_maps


def make_in_maps_v2(X, swap_mask):
    X = np.asarray(X, dtype=np.float32)
    swap_mask = np.asarray(swap_mask).astype(bool)
    b, c, t = X.shape
    half = c // 2

    # source channel for output channels 0..31 (stays within 0..31)
    cidx = np.arange(half, dtype=np.int32)
    mask_c = np.repeat(swap_mask, 2, axis=1)  # [b, 32]
    perm = np.where(mask_c, cidx[None, :] ^ 1, cidx[None, :]).astype(np.int32)

    in_maps = []
    for m in range(M):
        xs = np.ascontiguousarray(X[m * BL : (m + 1) * BL])  # [BL, C, T]
        pm = perm[m * BL : (m + 1) * BL]  # [BL, 32]
        # flat source row for (local batch bl, out channel ch<32)
        rows = (np.arange(BL, dtype=np.int32)[:, None] * c + pm).reshape(-1)
        idxm = np.ascontiguousarray(rows.reshape(-1, P).T.astype(np.int32))
        in_maps.append({"x": xs, "idx": idxm})
    return in_maps


def make_in_maps_v4(X, swap_mask):
    X = np.asarray(X, dtype=np.float32)
    swap_mask = np.asarray(swap_mask).astype(bool)
    b, c, t = X.shape
    half = c // 2

    cidx = np.arange(half, dtype=np.int32)
    mask_c = np.repeat(swap_mask, 2, axis=1)
    perm = np.where(mask_c, cidx[None, :] ^ 1, cidx[None, :]).astype(np.int32)

    nchunk = BL * half // P
    bpc = P // half
    in_maps, init_outs = [], []
    for m in range(M):
        pm = perm[m * BL : (m + 1) * BL]
        rows = (np.arange(BL, dtype=np.int32)[:, None] * c + pm).reshape(-1)
        idxm = np.ascontiguousarray(rows.reshape(-1, P).T.astype(np.int32))
        in_maps.append({"idx": idxm})
        init_outs.append({"y": np.ascontiguousarray(X[m * BL : (m + 1) * BL])})
    return in_maps, init_outs


class _V4Result:
    def __init__(self, exec_time_ns=None):
        self.exec_time_ns = exec_time_ns
        self.mean_exec_time_ns = exec_time_ns


def _ntff_capture(output_dir, device_ids):
    """Self-contained NTFF capture via libaxon_pjrt.so (trace path only)."""
    import contextlib as _cl
    import ctypes

    lib = ctypes.CDLL("/opt/axon/libaxon_pjrt.so")
    lib.axon_start_nrt_profile.argtypes = [
        ctypes.POINTER(ctypes.c_int64),
        ctypes.c_size_t,
    ]
    lib.axon_start_nrt_profile.restype = ctypes.c_int64
    lib.axon_stop_nrt_profile.argtypes = [ctypes.c_char_p]
    lib.axon_stop_nrt_profile.restype = ctypes.c_int64

    @_cl.contextmanager
    def _hook():
        import jax

        jax.devices()
        ids = (ctypes.c_int64 * len(device_ids))(*device_ids)
        rc = lib.axon_start_nrt_profile(ids, len(device_ids))
        if rc != 0:
            raise RuntimeError(f"axon_start_nrt_profile rc={rc}")
        try:
            yield
        finally:
            n = lib.axon_stop_nrt_profile(str(output_dir).encode())
            print(f"profile: {n} file(s) in {output_dir}", file=sys.stderr)

    return _hook()


SPLIT = 4


def _run_v4(X, swap_mask, trace=False):
    assign = None
    if VERSION == 11:
        in_maps, init_outs, caps, assign = make_in_maps_v11(X, swap_mask)
        nc = build_bass_v11(caps, nbuf=min(len(caps), 6))
    elif VERSION == 9:
        in_maps, init_outs, nchunk = make_in_maps_v9(X, swap_mask, SPLIT)
        nc = build_bass_v9(nchunk, nbuf=min(nchunk, 6), split=SPLIT)
    elif VERSION == 8:
        in_maps, init_outs, nchunk = make_in_maps_v8(X, swap_mask, SPLIT)
        nc = build_bass_v8(nchunk, nbuf=min(nchunk, 6), split=SPLIT)
    elif VERSION == 7:
        in_maps, init_outs, nchunk = make_in_maps_v7(X, swap_mask)
        nc = build_bass_v7(nchunk, nbuf=min(nchunk, 6))
    elif VERSION == 6:
        nc = build_bass_v6()
        in_maps, init_outs = make_in_maps_v6(X, swap_mask)
    else:
        nc = build_bass_v5() if VERSION == 5 else build_bass_v4()
        in_maps, init_outs = make_in_maps_v4(X, swap_mask)
    nc.finalize()
    exec_time_ns = None
    if trace:
        import glob
        import os
        import tempfile

        neff_dir = tempfile.mkdtemp()
        with _ntff_capture(neff_dir, [0]):
            results = _run_pjrt_with_init(nc, in_maps, init_outs)
        ntffs = glob.glob(os.path.join(neff_dir, "*_body*.ntff"))
        if ntffs:
            import gauge.profiler
            from concourse.bass_utils import FishPath

            profile = gauge.profiler.Profile(
                profile_path=FishPath(neff_dir),
                kernel_dev_mode=True,
                profile_on_exit=False,
                bass_kernel=nc.m,
                offline_processing=True,
                fname="*_body*",
                metadata={"artifacts_path": f"local:{neff_dir}"},
            )
            pr = profile.to_perfetto(model_index=(0,))
            if pr:
                exec_time_ns = pr[0].exec_time_ns
            print(f"ntff json dir: {neff_dir}", file=sys.stderr)
    else:
        results = _run_pjrt_with_init(nc, in_maps, init_outs)
    if assign is not None:
        out = np.empty((B, C, T), dtype=np.float32)
        for m in range(M):
            out[assign[m]] = results[m]["y"]
    else:
        out = np.concatenate([r["y"] for r in results], axis=0)
    return out, _V4Result(exec_time_ns)


VERSION = 11


def run(X, swap_mask, **kw):
    if VERSION in (4, 5, 6, 7, 8, 9, 11):
        return _run_v4(X, swap_mask, trace=kw.get("trace", False))
    if VERSION == 2:
        nc = build_bass_v2()
        in_maps = make_in_maps_v2(X, swap_mask)
    else:
        nc = build_bass()
        in_maps = make_in_maps(X, swap_mask)
    if not nc.is_finalized():
        nc.finalize()
    res = run_bass_kernel_spmd(nc, in_maps, list(range(M)), **kw)
    out = np.concatenate(
        [r["y"].reshape(BL, C, T) for r in res.results], axis=0
    )
    return out, res


def kernel(X, swap_mask):
    out, _ = run(X, swap_mask)
    return out

